# revision 1
# baseline (speedup 1.0000x reference)
"""AlignedTripletLoss Trainium2 kernel (8 NeuronCores, symmetric block-pair
decomposition).

Math (matches reference.py):
  x_hat = x / (||x||_2 + 1e-12) per (image, part) row               [1024*8, 128]
  dist2[(a,i),(b,j)] = 2 - 2 * <x_hat_(a,i), x_hat_(b,j)>  (rows are unit norm,
      so the sq-norm terms are 1 up to ~1e-6; a +4e-4 bias keeps sqrt's argument
      positive on the diagonal despite float32r matmul rounding)
  t = tanh(0.5 * sqrt(dist2))
  dtw[a,b] = monotone (right/down) shortest path over the 8x8 grid t[i][j]
  ap = max over positives, an = min over negatives, loss = mean(relu(ap-an+0.3))

Sharding: dtw is symmetric (the DTW of the transposed cell grid is the mirrored
path set), so only unordered image-block pairs need computing. Core k computes
blocks (k, (k+d) mod 8) for d = 0..4 -- a uniform circulant cover of all 36
unordered pairs (d=4 pairs are computed twice; min/max mining makes duplicates
harmless). That is 640 columns per core instead of 1024 (37.5% less DTW work).
Each core row-mines its own anchors over its 640 columns AND, after a PE
transpose of each off-diagonal [128,128] dtw block, column-mines the partner
block's anchors. Per-core output is 10 partial min/max vectors; the host glue
combines partials per anchor (placement by core id) and takes the mean -- the
analogue of the sharding hint's final all-reduce.

The DTW row recurrence val[j] = min(val[j-1], up[j]) + t[i][j] is exactly DVE
tensor_tensor_scan(op0=min, op1=add) along the free axis, with a dummy element
between consecutive (a,b) pairs to reset the running state (data0 dummy -BIG
with data1 dummy +BIG resets to 0 on row 0; prior-row outputs regenerate the
reset for rows 1..7 automatically).

Perf notes:
 - pairwise dots run as float32r matmuls (1 cyc/col at free dim >= 256);
   inputs are rounded to f32r by the producing copies as walrus requires.
 - normalization scale is folded into the transpose: PE matmul against a
   gpsimd-built diag(1/norm) transposes and scales in one pass.
 - sqrt reads PSUM with a scattered (b,j) AP (free) and writes the scan layout
   in 32B runs (measured full rate); tanh runs in place on the same layout.
 - row buffers live in a persistent arena whose scan-reset dummies are memset
   exactly once.
"""

import numpy as np

N, M, D = 1024, 8, 128
MARGIN = 0.3
EPS = 1e-12
NCORES = 8
A = N // NCORES          # anchors per core (one image block)
NDIAG = 5                # circulant depth: blocks k..k+4
NCOL = NDIAG * A         # 640 columns per core
CBS = [256, 256, 128]    # column batches (blocks d=0,1 | d=2,3 | d=4)
CBMAX = 256
G = M + 1                # scan group: 1 dummy + 8 j-steps
BIG = 1e9
SQ_BIAS = 2.0 + 4e-4

_CACHE = {}


def _build_nc():
    import concourse.bacc as bacc
    import concourse.mybir as mybir
    import concourse.tile as tile
    from concourse.tile import add_dep_helper
    from concourse.masks import make_identity

    fp32 = mybir.dt.float32
    f32r = mybir.dt.float32r
    AF = mybir.ActivationFunctionType
    OP = mybir.AluOpType
    AX = mybir.AxisListType

    nc = bacc.Bacc("TRN2", target_bir_lowering=False, debug=False,
                   num_devices=NCORES)

    xa_in = nc.dram_tensor("xa", [A * M, D], fp32, kind="ExternalInput")
    xr_in = nc.dram_tensor("xr5", [NCOL * M, D], fp32, kind="ExternalInput")
    mop_in = nc.dram_tensor("m_own_pos", [A, NCOL], fp32, kind="ExternalInput")
    mon_in = nc.dram_tensor("m_own_neg", [A, NCOL], fp32, kind="ExternalInput")
    mtp_in = nc.dram_tensor("m_t_pos", [A, (NDIAG - 1) * A], fp32,
                            kind="ExternalInput")
    mtn_in = nc.dram_tensor("m_t_neg", [A, (NDIAG - 1) * A], fp32,
                            kind="ExternalInput")
    out_t = nc.dram_tensor("partials", [A, 10], fp32, kind="ExternalOutput")

    S = (NCOL * M) // 128   # 40 row-tiles of xr5
    SA = (A * M) // 128     # 8 row-tiles of xa
    NB = len(CBS)

    with tile.TileContext(nc) as tc:
        with tc.tile_pool(name="persist", bufs=1) as persist:
            xrT = persist.tile([128, NCOL, M], f32r)  # x_hat^T [d][b][j], b-major
            xTa = persist.tile([128, M, A], f32r)     # -2*x_hat_anchor^T [d][i][a]
            mop = persist.tile([128, NCOL], fp32)
            mon = persist.tile([128, NCOL], fp32)
            mtp = persist.tile([128, (NDIAG - 1) * A], fp32)
            mtn = persist.tile([128, (NDIAG - 1) * A], fp32)
            up0 = persist.tile([128, CBMAX * G], fp32)
            biasT = persist.tile([128, 1], fp32)
            dtwc = persist.tile([128, NCOL], fp32)    # compact dtw row block
            apacc = persist.tile([128, NB], fp32)
            anacc = persist.tile([128, NB], fp32)
            pout = persist.tile([128, 10], fp32)
            ident = persist.tile([128, 128], fp32)
            RSLOT = 10
            arena = persist.tile([128, RSLOT, CBMAX, G], fp32)

            nc.sync.dma_start(mop[:], mop_in[:])
            nc.sync.dma_start(mon[:], mon_in[:])
            nc.sync.dma_start(mtp[:], mtp_in[:])
            nc.sync.dma_start(mtn[:], mtn_in[:])
            nc.gpsimd.memset(biasT[:], SQ_BIAS)
            up0v = up0.rearrange("p (c g) -> p c g", g=G)
            nc.gpsimd.memset(up0v[:, :, 0:1], -BIG)
            nc.gpsimd.memset(up0v[:, :, 1:G], BIG)
            nc.gpsimd.memset(arena[:, :, :, 0:1], BIG)
            make_identity(nc, ident[:])

            # ---------- setup: normalize + transpose (scale fused via diag) ----
            with (
                tc.tile_pool(name="setup", bufs=1) as setup,
                tc.tile_pool(name="chunk", bufs=2) as chunk,
                tc.tile_pool(name="dgp", bufs=3) as dgp,
                tc.tile_pool(name="tpsum", bufs=2, space="PSUM") as tpsum,
            ):
                def norm_rn(src_dram, n_tiles, neg2, tagp, src_ap=None):
                    """Rows p-outer (row r = p*n_tiles + s); rn = 1/(||row||+eps)."""
                    xr = setup.tile([128, n_tiles, D], fp32, tag=f"xr{tagp}")
                    if src_ap is None:
                        src_ap = src_dram.rearrange("(p s) d -> p s d", p=128)
                        nc.sync.dma_start(xr[:], src_ap)
                    else:
                        xrv = xr.rearrange("p (blk s) d -> p blk s d", s=M)
                        nblk = n_tiles // M
                        for b0 in range(0, nblk, 2):
                            b1 = min(b0 + 2, nblk)
                            nc.sync.dma_start(
                                xrv[:, b0:b1, :, :], src_ap[:, b0:b1, :, :])
                    n2 = setup.tile([128, n_tiles], fp32, tag=f"n2{tagp}")
                    nrm = setup.tile([128, n_tiles], fp32, tag=f"nr{tagp}")
                    rn = setup.tile([128, n_tiles], fp32, tag=f"rn{tagp}")
                    for g in range(0, n_tiles, 16):
                        CH = min(16, n_tiles - g)
                        x2 = chunk.tile([128, 16, D], fp32, tag="x2c")
                        nc.scalar.activation(
                            x2[:, :CH, :], xr[:, g:g + CH, :], AF.Square)
                        nc.vector.tensor_reduce(
                            n2[:, g:g + CH], x2[:, :CH, :], axis=AX.X, op=OP.add)
                        nc.scalar.activation(
                            nrm[:, g:g + CH], n2[:, g:g + CH], AF.Sqrt)
                        nc.vector.tensor_scalar_add(
                            nrm[:, g:g + CH], nrm[:, g:g + CH], EPS)
                        nc.vector.reciprocal(
                            rn[:, g:g + CH], nrm[:, g:g + CH])
                        if neg2:
                            nc.vector.tensor_scalar_mul(
                                rn[:, g:g + CH], rn[:, g:g + CH], -2.0)
                    return xr, rn

                def diag4(rn, s0):
                    dgc = dgp.tile([128, 4, 128], fp32, tag="dgc")
                    for jj in range(4):
                        nc.gpsimd.affine_select(
                            out=dgc[:, jj, :],
                            in_=rn[:, s0 + jj:s0 + jj + 1].to_broadcast((128, 128)),
                            compare_op=OP.is_equal, fill=0.0, base=0,
                            pattern=[[-1, 128]], channel_multiplier=1)
                    return dgc

                xra, rna = norm_rn(xa_in, SA, neg2=True, tagp="a")
                for half in range(2):
                    dgc = diag4(rna, 4 * half)
                    pt = tpsum.tile([128, 4, 128], fp32, tag="tp")
                    for jj in range(4):
                        s = 4 * half + jj
                        nc.tensor.matmul(
                            pt[:, jj, :], lhsT=xra[:, s, :],
                            rhs=dgc[:, jj, :], start=True, stop=True)
                    # tile s holds rows r = p*8+s -> (a=p, i=s)
                    dst = xTa[:, 4 * half:4 * half + 4, :]
                    nc.scalar.activation(dst, pt[:], AF.Copy)

                # xr5 laid out per block: tile t = blk*8 + s holds rows
                # blk*1024 + p*8 + s -> (col = blk*128 + p, j = s), so early
                # blocks complete first and batch-0 matmuls start sooner.
                xr, rn = norm_rn(
                    xr_in, S, neg2=False, tagp="x",
                    src_ap=xr_in.rearrange(
                        "(blk p s) d -> p blk s d", p=128, s=M))
                for blk in range(NDIAG):
                    for half in range(2):
                        dgc = diag4(rn, 8 * blk + 4 * half)
                        pt = tpsum.tile([128, 4, 128], fp32, tag="tp")
                        for jj in range(4):
                            s = 8 * blk + 4 * half + jj
                            nc.tensor.matmul(
                                pt[:, jj, :], lhsT=xr[:, s, :],
                                rhs=dgc[:, jj, :], start=True, stop=True)
                        dst = xrT[:, blk * A:(blk + 1) * A,
                                  4 * half:4 * half + 4]
                        nc.scalar.activation(
                            dst, pt.rearrange("d j b -> d b j"), AF.Copy)


            # ---------- main loop ----------
            with (
                tc.tile_pool(name="valsp", bufs=3) as valsp,
                tc.tile_pool(name="mtmp", bufs=6) as mtmp,
                tc.tile_pool(name="mpsum", bufs=2, space="PSUM") as mpsum,
            ):
                prev_tanh_last = None
                slot = 0
                col0 = 0
                tblocks = [[1], [2, 3], [4]]
                for n in range(NB):
                    CB = CBS[n]
                    sd = []
                    sqrt_insts = []
                    for i in range(M):
                        pp = mpsum.tile([128, M, CBMAX], fp32, tag="pp")
                        for j in range(M):
                            nc.tensor.matmul(
                                pp[:, j, :CB], lhsT=xTa[:, i, :],
                                rhs=xrT[:, col0:col0 + CB, j],
                                start=True, stop=True)
                        buf = arena[:, slot, :CB, :]
                        slot = (slot + 1) % RSLOT
                        # read PSUM scattered in (b, j) order; write 32B runs
                        inst = nc.scalar.activation(
                            buf[:, :, 1:G],
                            pp[:, :, :CB].rearrange("p j b -> p b j"),
                            AF.Sqrt, bias=biasT[:, 0:1])
                        if prev_tanh_last is not None:
                            add_dep_helper(inst.ins, prev_tanh_last.ins,
                                           sync=False,
                                           reason="ACT table batch order")
                        sqrt_insts.append(inst)
                        sd.append(buf)
                    for i in range(M):
                        v = sd[i][:, :, 1:G]
                        t_inst = nc.scalar.activation(v, v, AF.Tanh, scale=0.5)
                        add_dep_helper(t_inst.ins, sqrt_insts[-1].ins,
                                       sync=False,
                                       reason="ACT table batch order")
                        prev_tanh_last = t_inst
                    prev_vals = None
                    for i in range(M):
                        vt = valsp.tile([128, CBMAX * G], fp32, tag="vals")
                        d0 = up0[:, :CB * G] if i == 0 else prev_vals[:, :CB * G]
                        nc.vector.tensor_tensor_scan(
                            vt[:, :CB * G], d0,
                            sd[i].rearrange("p c g -> p (c g)"),
                            0.0, OP.min, OP.add)
                        prev_vals = vt
                    dtw = prev_vals.rearrange(
                        "p (c g) -> p c g", g=G)[:, :CB, M:M + 1]
                    dtw = dtw.rearrange("p c o -> p (c o)")
                    # compact copy (feeds block transposes + mining)
                    nc.vector.tensor_copy(dtwc[:, col0:col0 + CB], dtw)
                    tp = mtmp.tile([128, CBMAX], fp32, tag="tp")
                    nc.vector.tensor_tensor(
                        tp[:, :CB], dtwc[:, col0:col0 + CB],
                        mop[:, col0:col0 + CB], OP.add)
                    nc.vector.tensor_reduce(
                        apacc[:, n:n + 1], tp[:, :CB], axis=AX.X, op=OP.max)
                    tn = mtmp.tile([128, CBMAX], fp32, tag="tn")
                    nc.vector.tensor_tensor(
                        tn[:, :CB], dtwc[:, col0:col0 + CB],
                        mon[:, col0:col0 + CB], OP.add)
                    nc.vector.tensor_reduce(
                        anacc[:, n:n + 1], tn[:, :CB], axis=AX.X, op=OP.min)
                    col0 += CB

                # own-anchor partials -> pout cols 0 (an), 1 (ap)
                nc.vector.tensor_reduce(
                    pout[:, 0:1], anacc[:], axis=AX.X, op=OP.min)
                nc.vector.tensor_reduce(
                    pout[:, 1:2], apacc[:], axis=AX.X, op=OP.max)

                # transposed blocks d=1..4: partner anchors over our columns
                for d in range(1, NDIAG):
                    ptp = mpsum.tile([128, 128], fp32, tag="pp")
                    nc.tensor.transpose(
                        ptp[:], dtwc[:, d * A:(d + 1) * A], ident[:])
                    tb = mtmp.tile([128, 128], fp32, tag="tb")
                    nc.vector.tensor_copy(tb[:], ptp[:])
                    tpp = mtmp.tile([128, 128], fp32, tag="tpp")
                    nc.vector.tensor_tensor(
                        tpp[:], tb[:], mtp[:, (d - 1) * A:d * A], OP.add)
                    nc.vector.tensor_reduce(
                        pout[:, 2 * d + 1:2 * d + 2], tpp[:],
                        axis=AX.X, op=OP.max)
                    nc.vector.tensor_tensor(
                        tpp[:], tb[:], mtn[:, (d - 1) * A:d * A], OP.add)
                    nc.vector.tensor_reduce(
                        pout[:, 2 * d:2 * d + 1], tpp[:],
                        axis=AX.X, op=OP.min)

                nc.sync.dma_start(out_t[:], pout[:])

    nc.compile()
    return nc


def _get_nc():
    if "nc" not in _CACHE:
        _CACHE["nc"] = _build_nc()
    return _CACHE["nc"]


def kernel(inputs, labels, _trace=False, _trace_cores=None):
    from concourse.bass_utils import run_bass_kernel_spmd

    x = np.ascontiguousarray(np.asarray(inputs, dtype=np.float32)).reshape(N * M, D)
    lab = np.asarray(labels)

    nc = _get_nc()
    in_maps = []
    for c in range(NCORES):
        blocks = [(c + d) % NCORES for d in range(NDIAG)]
        col_img = np.concatenate([np.arange(b * A, (b + 1) * A) for b in blocks])
        row_img = np.arange(c * A, (c + 1) * A)
        xr5 = np.ascontiguousarray(
            x.reshape(N, M, D)[col_img].reshape(NCOL * M, D))
        xa = np.ascontiguousarray(x[c * A * M:(c + 1) * A * M])
        eq_own = lab[row_img][:, None] == lab[col_img][None, :]
        m_own_pos = np.where(eq_own, np.float32(0.0), np.float32(-1e30))
        m_own_neg = np.where(eq_own, np.float32(1e30), np.float32(0.0))
        # transposed blocks: anchors = block (c+d)%8, cols = block c images
        mtp_l, mtn_l = [], []
        for d in range(1, NDIAG):
            arow = lab[np.arange(blocks[d] * A, (blocks[d] + 1) * A)]
            eq_t = arow[:, None] == lab[row_img][None, :]
            mtp_l.append(np.where(eq_t, np.float32(0.0), np.float32(-1e30)))
            mtn_l.append(np.where(eq_t, np.float32(1e30), np.float32(0.0)))
        in_maps.append({
            "xa": xa,
            "xr5": xr5,
            "m_own_pos": np.ascontiguousarray(m_own_pos.astype(np.float32)),
            "m_own_neg": np.ascontiguousarray(m_own_neg.astype(np.float32)),
            "m_t_pos": np.ascontiguousarray(
                np.concatenate(mtp_l, axis=1).astype(np.float32)),
            "m_t_neg": np.ascontiguousarray(
                np.concatenate(mtn_l, axis=1).astype(np.float32)),
        })
    res = run_bass_kernel_spmd(
        nc, in_maps, core_ids=list(range(NCORES)), trace=_trace,
        trace_cores=_trace_cores)
    if _trace:
        _CACHE["last_results"] = res

    # host glue: combine per-core min/max partials per anchor block
    an_all = np.full((NCORES, A), np.inf, dtype=np.float32)
    ap_all = np.full((NCORES, A), -np.inf, dtype=np.float32)
    for c in range(NCORES):
        p = res.results[c]["partials"]  # [A, 10]
        for d in range(NDIAG):
            blk = (c + d) % NCORES
            an_all[blk] = np.minimum(an_all[blk], p[:, 2 * d])
            ap_all[blk] = np.maximum(ap_all[blk], p[:, 2 * d + 1])
    loss_vec = np.maximum(
        ap_all.reshape(-1) - an_all.reshape(-1) + np.float32(MARGIN),
        np.float32(0.0))
    return np.asarray(loss_vec.mean(), dtype=np.float32)



# revision 2
# speedup vs baseline: 1.0059x; 1.0059x over previous
"""AlignedTripletLoss Trainium2 kernel v2 (8 cores, fp16 wavefront DTW).

Math (matches reference.py):
  x_hat = x / ||x||_2 per (image, part) row
  c[(a,i),(b,j)] = <x_hat_(a,i), x_hat_(b,j)>;  d = sqrt(2+delta - 2c)
  t = tanh(0.5*d);  dtw[a,b] = monotone min-path over the 8x8 grid t[i][j]
  ap = max over positives, an = min over negatives, loss = mean(relu(ap-an+0.3))

Design vs v1 (the scan kernel):
 - fp16 end to end: features, xrT, T values, DTW state. Validated offline:
   rel err ~1.1e-3 with delta=4e-3 (keeps sqrt args positive).
 - T is stored DIAG-SLOT-MAJOR [p, slot=8(i+j)+i, col]: every DTW wavefront
   operand and every tanh instruction is a fully packed len*CB fp16 run
   (DVE 2x mode, measured 0.556 ns/elem; ACT full rate 0.87 ns/elem).
   The serial tensor_tensor_scan (2.15 ns/elem, no fast mode) is gone.
 - DTW = 15 wavefront steps of tensor_tensor min+add on DVE. Slot 0 of the
   V ping-pong buffers is a permanent +BIG pad (bottom boundary); the top
   boundary cell (i=s, j=0) is an explicit 1-slot add, so no per-batch
   memsets are needed and stale deep slots are never read.
 - ACT: sqrt reads each i's PSUM [p,(j,c)] and scatters to slots 9i+8j
   (runs of CB fp16 = 512B, full rate), with scale=-2 / bias=2+delta folding
   the -2c and the bias in one pass. tanh runs per diagonal in place.
 - matmuls are fp16 (1 cyc/row at any free size); lhsT is xrT's own block
   slice directly (no separate -2-scaled anchor copy).
 - host prepares per-core fp16 arrays in DMA/transpose-friendly layouts and
   fp16 +-3e4 label masks (much smaller than v1's fp32 +-1e30 masks).
Sharding: same symmetric circulant block cover as v1 (core k owns blocks
k..k+4; transposed mining covers the partner orientation; host combines
per-anchor min/max partials).
"""

import numpy as np

N, M, D = 1024, 8, 128
MARGIN = 0.3
NCORES = 8
A = N // NCORES          # 128 anchors per core
NDIAG = 5
NCOL = NDIAG * A         # 640 columns per core
CBS = [256, 256, 128]    # column batches (blocks 0,1 | 2,3 | 4)
CBMAX = 256
NT = NCOL // 16          # 40 row-tiles of xr5a (16 cols x 8 parts each)
BIG = 30000.0
DELTA = 4e-3
SQ_BIAS = 2.0 + DELTA

_CACHE = {}


def _build_nc():
    import concourse.bacc as bacc
    import concourse.mybir as mybir
    import concourse.tile as tile
    from concourse.tile import add_dep_helper
    from concourse.masks import make_identity

    fp32 = mybir.dt.float32
    fp16 = mybir.dt.float16
    AF = mybir.ActivationFunctionType
    OP = mybir.AluOpType
    AX = mybir.AxisListType

    nc = bacc.Bacc("TRN2", target_bir_lowering=False, debug=False,
                   num_devices=NCORES)

    xr_in = nc.dram_tensor("xr5a", [128, NT, D], fp16, kind="ExternalInput")
    rn_in = nc.dram_tensor("rn16", [128, NT], fp16, kind="ExternalInput")
    mop_in = nc.dram_tensor("m_own_pos", [A, NCOL], fp16, kind="ExternalInput")
    mon_in = nc.dram_tensor("m_own_neg", [A, NCOL], fp16, kind="ExternalInput")
    mtp_in = nc.dram_tensor("m_t_pos", [A, (NDIAG - 1) * A], fp16,
                            kind="ExternalInput")
    mtn_in = nc.dram_tensor("m_t_neg", [A, (NDIAG - 1) * A], fp16,
                            kind="ExternalInput")
    out_t = nc.dram_tensor("partials", [A, 10], fp32, kind="ExternalOutput")

    NB = len(CBS)

    with tile.TileContext(nc) as tc:
        with tc.tile_pool(name="persist", bufs=1) as persist:
            xrraw = persist.tile([128, NT, D], fp16)
            xrT = persist.tile([128, M, NCOL], fp16)   # [d][j][col]
            rn = persist.tile([128, NT], fp16)
            mop = persist.tile([128, NCOL], fp16)
            mon = persist.tile([128, NCOL], fp16)
            mtp = persist.tile([128, (NDIAG - 1) * A], fp16)
            mtn = persist.tile([128, (NDIAG - 1) * A], fp16)
            Tar = persist.tile([128, 2, 120, CBMAX], fp16)  # slot-major T
            Va = persist.tile([128, 9, CBMAX], fp16)
            Vb = persist.tile([128, 9, CBMAX], fp16)
            dtwc = persist.tile([128, NCOL], fp32)
            ident = persist.tile([128, 128], fp32)
            apacc = persist.tile([128, NB], fp32)
            anacc = persist.tile([128, NB], fp32)
            pout = persist.tile([128, 10], fp32)
            biasT = persist.tile([128, 1], fp32)
            sclT = persist.tile([128, 1], fp32)
            pw = persist.tile([128, M, CBMAX], fp16)
            pnum = persist.tile([128, M, CBMAX], fp16)

            nc.sync.dma_start(rn[:], rn_in[:])
            for b in range(NDIAG):
                nc.sync.dma_start(xrraw[:, 8 * b:8 * b + 8, :],
                                  xr_in[:, 8 * b:8 * b + 8, :])
            nc.sync.dma_start(mop[:], mop_in[:])
            nc.sync.dma_start(mon[:], mon_in[:])
            nc.sync.dma_start(mtp[:], mtp_in[:])
            nc.sync.dma_start(mtn[:], mtn_in[:])
            nc.gpsimd.memset(Va[:, 0:1, :], BIG)
            nc.gpsimd.memset(Vb[:, 0:1, :], BIG)
            # fp16 identity for PE transposes of dtw blocks
            make_identity(nc, ident[:])
            warm = persist.tile([128, 1], fp32)
            wmm = persist.tile([128, 512], fp16)
            nc.gpsimd.memset(wmm[:], 0.5)
            nc.gpsimd.memset(biasT[:], SQ_BIAS)
            nc.gpsimd.memset(sclT[:], -2.0)

            act_chain = [None]

            def act(out, in_, func, **kw):
                inst = nc.scalar.activation(out, in_, func, **kw)
                if act_chain[0] is not None:
                    add_dep_helper(inst.ins, act_chain[0].ins, sync=False,
                                   reason="ACT table batch order")
                act_chain[0] = inst
                return inst

            with (
                tc.tile_pool(name="dgp", bufs=3) as dgp,
                tc.tile_pool(name="mpsum", bufs=2, space="PSUM") as mpsum,
                tc.tile_pool(name="mtmp", bufs=4) as mtmp,
            ):
                # hoists the sqrt table load to t~0 (otherwise it glues to
                # the first real sqrt ~20us in)
                act(warm[:], biasT[:], AF.Sqrt)

                # -------- transpose + scale fold: per block, 8 tiles -----
                def do_transposes(b):
                    # two groups of 4 tiles per block, sharing one psum slot
                    for half in range(2):
                        t0 = 8 * b + 4 * half
                        dgc = dgp.tile([128, 4, 128], fp16, tag="dgc")
                        nc.gpsimd.affine_select(
                            out=dgc[:],
                            in_=rn[:, t0:t0 + 4].to_broadcast((128, 4, 128)),
                            compare_op=OP.is_equal, fill=0.0, base=0,
                            pattern=[[0, 4], [-1, 128]], channel_multiplier=1)
                        pt = mpsum.tile([128, M, CBMAX], fp32, tag="pp")
                        for q in range(4):
                            t = 8 * b + 4 * half + q
                            nc.tensor.matmul(
                                pt[:, q, :128], lhsT=xrraw[:, t, :],
                                rhs=dgc[:, q, :], start=True, stop=True)
                        # one copy per 4-tile group:
                        # psum [d, q, j, cc] -> xrT[d, j, col0 + q*16 + cc]
                        t0 = 8 * b + 4 * half
                        col0 = b * A + (t0 % 8) * 16
                        nc.vector.tensor_copy(
                            xrT[:, :, col0:col0 + 64].rearrange(
                                "p j (q c) -> p q j c", c=16),
                            pt[:, 0:4, :128].rearrange(
                                "p q (j c) -> p q j c", c=16))

                do_transposes(0)
                do_transposes(1)

                # ---------------- main batches ----------------
                COL0 = [0]
                for nb in range(NB):
                    COL0.append(COL0[-1] + CBS[nb])
                TSLOT = [0, 1, 0]

                def mm_sqrt(nb, extras=None):
                    """matmuls (j-pairs, 512 free) + per-i sqrt into T slot.
                    extras: {i: callable} run after i's emission (interleaves
                    other same-table ACT work / PE work into slack)."""
                    CB = CBS[nb]
                    col0 = COL0[nb]
                    T = Tar[:, TSLOT[nb], :, :CB]
                    for i in range(M):
                        pp = mpsum.tile([128, M, CBMAX], fp32, tag="pp")
                        for jp in range(0, M, 2):
                            nc.tensor.matmul(
                                pp[:, jp:jp + 2, :CB],
                                lhsT=xrT[:, i, 0:128],
                                rhs=xrT[:, jp:jp + 2, col0:col0 + CB],
                                start=True, stop=True)
                        act(T[:, 9 * i:9 * i + 57:8, :],
                            pp[:, :, :CB], AF.Sqrt,
                            scale=sclT[:, 0:1], bias=biasT[:, 0:1])
                        if extras and i in extras:
                            extras[i]()

                def tanh_batch(nb, skip7=False):
                    CB = CBS[nb]
                    T = Tar[:, TSLOT[nb], :, :CB]
                    for s in range(15):
                        if skip7 and s == 7:
                            continue
                        i_min, i_max = max(0, s - 7), min(7, s)
                        v = T[:, 8 * s + i_min:8 * s + i_max + 1, :]
                        act(v, v, AF.Tanh, scale=0.5)

                def tanh7_dve(nb):
                    """tanh(d/2) ~= d(60+d^2)/(120+12d^2) for diagonal 7,
                    on the otherwise-idle DVE (abs err <= 3.1e-4 on d in
                    [0,2], below fp16 noise)."""
                    CB = CBS[nb]
                    dd = Tar[:, TSLOT[nb], 56:64, :CB]
                    w = pw[:, :, :CB]
                    num = pnum[:, :, :CB]
                    with nc.allow_low_precision(
                            reason="fp16 Pade tanh, validated offline"):
                        nc.vector.tensor_tensor(w, dd, dd, OP.mult)
                        nc.vector.scalar_tensor_tensor(
                            num, w, 60.0, dd, OP.add, OP.mult)
                        nc.vector.tensor_scalar(
                            w, w, 12.0, 120.0, OP.mult, OP.add)
                        nc.vector.reciprocal(w, w)
                        nc.vector.tensor_tensor(dd, num, w, OP.mult)

                def pyramid_mine(nb, nsplit=1):
                    CB = CBS[nb]
                    col0 = COL0[nb]
                    T = Tar[:, TSLOT[nb], :, :CB]
                    # nsplit independent column chains, interleaved so the
                    # serial step-to-step semaphore latency overlaps
                    H = CB // nsplit
                    ranges = [(h * H, (h + 1) * H) for h in range(nsplit)]
                    Vp, Vc = Va, Vb
                    for (c0, c1) in ranges:
                        nc.vector.tensor_copy(
                            Vc[:, 1:2, c0:c1], T[:, 0:1, c0:c1])
                    Vp, Vc = Vc, Vp
                    for s in range(1, 15):
                        i_min, i_max = max(0, s - 7), min(7, s)
                        for (c0, c1) in ranges:
                            if s <= 7:
                                # cells i in [0, s-1]; slot-0 pad (BIG)
                                # covers the missing "up" of i=0
                                nc.vector.tensor_tensor(
                                    Vc[:, 1:s + 1, c0:c1],
                                    Vp[:, 0:s, c0:c1],
                                    Vp[:, 1:s + 1, c0:c1], OP.min)
                                nc.vector.tensor_tensor(
                                    Vc[:, 1:s + 1, c0:c1],
                                    Vc[:, 1:s + 1, c0:c1],
                                    T[:, 8 * s:8 * s + s, c0:c1], OP.add)
                                # top cell i=s (j=0): up only
                                nc.vector.tensor_tensor(
                                    Vc[:, s + 1:s + 2, c0:c1],
                                    Vp[:, s:s + 1, c0:c1],
                                    T[:, 9 * s:9 * s + 1, c0:c1], OP.add)
                            else:
                                k0, k1 = i_min + 1, i_max + 2
                                nc.vector.tensor_tensor(
                                    Vc[:, k0:k1, c0:c1],
                                    Vp[:, k0 - 1:k1 - 1, c0:c1],
                                    Vp[:, k0:k1, c0:c1], OP.min)
                                nc.vector.tensor_tensor(
                                    Vc[:, k0:k1, c0:c1],
                                    Vc[:, k0:k1, c0:c1],
                                    T[:, 8 * s + i_min:8 * s + i_max + 1,
                                      c0:c1], OP.add)
                        Vp, Vc = Vc, Vp
                    dtw = Vp[:, 8:9, :CB].rearrange("p o c -> p (o c)")

                    nc.vector.tensor_copy(dtwc[:, col0:col0 + CB], dtw)
                    tp = mtmp.tile([128, CBMAX], fp16, tag="tp")
                    nc.vector.tensor_tensor(
                        tp[:, :CB], dtw, mop[:, col0:col0 + CB], OP.add)
                    nc.vector.tensor_reduce(
                        apacc[:, nb:nb + 1], tp[:, :CB], axis=AX.X, op=OP.max)
                    tn = mtmp.tile([128, CBMAX], fp16, tag="tn")
                    nc.vector.tensor_tensor(
                        tn[:, :CB], dtw, mon[:, col0:col0 + CB], OP.add)
                    nc.vector.tensor_reduce(
                        anacc[:, nb:nb + 1], tn[:, :CB], axis=AX.X, op=OP.min)

                def tmine(d):
                    """partner-anchor mining via PE transpose of block d."""
                    ptp = mpsum.tile([128, M, CBMAX], fp32, tag="pp")
                    nc.tensor.transpose(
                        ptp[:, 0, :128], dtwc[:, d * A:(d + 1) * A],
                        ident[:])
                    tb = mtmp.tile([128, 128], fp16, tag="tb")
                    nc.vector.tensor_copy(tb[:], ptp[:, 0, :128])
                    tpp = mtmp.tile([128, 128], fp16, tag="tpp")
                    nc.vector.tensor_tensor(
                        tpp[:], tb[:], mtp[:, (d - 1) * A:d * A], OP.add)
                    nc.vector.tensor_reduce(
                        pout[:, 2 * d + 1:2 * d + 2], tpp[:],
                        axis=AX.X, op=OP.max)
                    nc.vector.tensor_tensor(
                        tpp[:], tb[:], mtn[:, (d - 1) * A:d * A], OP.add)
                    nc.vector.tensor_reduce(
                        pout[:, 2 * d:2 * d + 1], tpp[:],
                        axis=AX.X, op=OP.min)

                # alternating table phases: block 2-4 transposes and the
                # next batch's matmuls run on the otherwise-idle PE during
                # tanh phases; transposed mining stays at the very end so
                # the PE stream never waits on a pyramid mid-kernel.
                mm_sqrt(0)
                tanh7_dve(0)
                tanh_batch(0, skip7=True)
                do_transposes(2)
                do_transposes(3)
                mm_sqrt(1)
                pyramid_mine(0)
                tanh7_dve(1)
                tanh_batch(1, skip7=True)
                do_transposes(4)
                mm_sqrt(2)
                pyramid_mine(1)
                tmine(1)
                tmine(2)
                tmine(3)
                tanh_batch(2)
                pyramid_mine(2, nsplit=2)
                tmine(4)

                nc.vector.tensor_reduce(
                    pout[:, 0:1], anacc[:], axis=AX.X, op=OP.min)
                nc.vector.tensor_reduce(
                    pout[:, 1:2], apacc[:], axis=AX.X, op=OP.max)
                nc.sync.dma_start(out_t[:], pout[:])

    nc.compile()
    return nc


def _get_nc():
    if "nc" not in _CACHE:
        _CACHE["nc"] = _build_nc()
    return _CACHE["nc"]


def kernel(inputs, labels, _trace=False, _trace_cores=None):
    from concourse.bass_utils import run_bass_kernel_spmd

    x16 = np.asarray(inputs, dtype=np.float16).reshape(N, M, D)
    lab = np.asarray(labels)

    nc = _get_nc()
    in_maps = []
    for c in range(NCORES):
        blocks = [(c + d) % NCORES for d in range(NDIAG)]
        col_img = np.concatenate([np.arange(b * A, (b + 1) * A) for b in blocks])
        row_img = np.arange(c * A, (c + 1) * A)
        # xr5a[p, blk*8+chunk, :] = x16[img(blk,chunk,p%16), p//16, :]
        # with img = blk*128 + chunk*16 + cc and p = j*16 + cc
        arr = np.empty((128, NT, D), dtype=np.float16)
        for bi, b in enumerate(blocks):
            sub = x16[b * A:(b + 1) * A]          # [128 imgs, 8, 128]
            arr[:, bi * 8:(bi + 1) * 8, :] = (
                sub.reshape(8, 16, M, D).transpose(2, 1, 0, 3)
                .reshape(128, 8, D))
        eq_own = lab[row_img][:, None] == lab[col_img][None, :]
        m_own_pos = np.where(eq_own, np.float16(0.0), np.float16(-BIG))
        m_own_neg = np.where(eq_own, np.float16(BIG), np.float16(0.0))
        mtp_l, mtn_l = [], []
        for d in range(1, NDIAG):
            arow = lab[np.arange(blocks[d] * A, (blocks[d] + 1) * A)]
            eq_t = arow[:, None] == lab[row_img][None, :]
            mtp_l.append(np.where(eq_t, np.float16(0.0), np.float16(-BIG)))
            mtn_l.append(np.where(eq_t, np.float16(BIG), np.float16(0.0)))
        nrm = np.linalg.norm(arr.astype(np.float32), axis=2)
        rn16 = (1.0 / nrm).astype(np.float16)
        in_maps.append({
            "xr5a": np.ascontiguousarray(arr),
            "rn16": np.ascontiguousarray(rn16),
            "m_own_pos": np.ascontiguousarray(m_own_pos.astype(np.float16)),
            "m_own_neg": np.ascontiguousarray(m_own_neg.astype(np.float16)),
            "m_t_pos": np.ascontiguousarray(
                np.concatenate(mtp_l, axis=1).astype(np.float16)),
            "m_t_neg": np.ascontiguousarray(
                np.concatenate(mtn_l, axis=1).astype(np.float16)),
        })
    res = run_bass_kernel_spmd(
        nc, in_maps, core_ids=list(range(NCORES)), trace=_trace,
        trace_cores=_trace_cores)
    if _trace:
        _CACHE["last_results"] = res

    an_all = np.full((NCORES, A), np.inf, dtype=np.float32)
    ap_all = np.full((NCORES, A), -np.inf, dtype=np.float32)
    for c in range(NCORES):
        p = res.results[c]["partials"]  # [A, 10]
        for d in range(NDIAG):
            blk = (c + d) % NCORES
            an_all[blk] = np.minimum(an_all[blk], p[:, 2 * d])
            ap_all[blk] = np.maximum(ap_all[blk], p[:, 2 * d + 1])
    loss_vec = np.maximum(
        ap_all.reshape(-1) - an_all.reshape(-1) + np.float32(MARGIN),
        np.float32(0.0))
    return np.asarray(loss_vec.mean(), dtype=np.float32)
